# revision 5
# baseline (speedup 1.0000x reference)
"""FBCritic embedding-lookup kernel for 8 Trainium2 NeuronCores.

Math (reference):
    fwd_idx = clip(obs)*10 + clip(act)            # [8192]
    bwd_idx = clip(fobs)*10 + clip(fact)          # [8192]
    F = W_f[fwd_idx]                              # [8192, 64]
    B = W_b[bwd_idx]                              # [8192, 64]
    out = F @ B.T                                 # [8192, 8192]

Sharding: data-parallel over the forward batch. Core c computes output rows
[c*1024, (c+1)*1024) against all 8192 backward columns.

Per core: batched indirect DMAs gather 512 table rows per instruction
(offset AP [128, 8] - 8 row-indices per destination partition, each index
pulling one 256B table row), a gpsimd copy casts the gathered f32 rows to
bf16, PE transposes produce [64, n*128] bf16 operands in PSUM (transpose
output dtype follows its input, so the PSUM tile is bf16 and the
PSUM->SBUF operand copy runs in the DVE 16-bit fast path), and bf16
matmuls accumulate f32 into PSUM. Strip evacuation (PSUM f32 -> SBUF bf16)
alternates between the vector and scalar engines, and [128, 2048] bf16
strips (4KB per partition row) stream to HBM on the sync HWDGE queue.

The output lands in HBM as bf16 (halving the dominant write traffic) and
is upcast to f32 on the host after the gather of per-core results.
"""

import numpy as np

NUM_OBS = 100000
NUM_ACT = 10
V = NUM_OBS * NUM_ACT  # 1_000_000 table rows
D = 64                 # repr dim
B = 8192               # batch
N_CORES = 8
M = B // N_CORES       # 1024 output rows per core
P = 128                # partitions

_CACHE = {}
TRACE = False
LAST_RESULT = None


def _build_nc():
    import concourse.bass as bass
    import concourse.tile as tile
    from concourse import bacc, mybir
    from concourse.masks import make_identity

    f32 = mybir.dt.float32
    bf16 = mybir.dt.bfloat16
    i32 = mybir.dt.int32

    nc = bacc.Bacc("TRN2", target_bir_lowering=False, debug=False)

    wf = nc.dram_tensor("wf", [V, D], f32, kind="ExternalInput").ap()
    wb = nc.dram_tensor("wb", [V, D], f32, kind="ExternalInput").ap()
    idxf_d = nc.dram_tensor("idxf", [P, M // P], i32, kind="ExternalInput").ap()
    idxb_d = nc.dram_tensor("idxb", [P, B // P], i32, kind="ExternalInput").ap()
    out_d = nc.dram_tensor("out", [M, B], bf16, kind="ExternalOutput").ap()

    GF = M // P     # 8 forward 128-row groups
    GB = B // P     # 64 backward 128-row groups
    SG = 8          # groups per indirect-DMA gather slice (512 rows)
    CH = 2048       # output column chunk / strip width (4KB bf16 rows)
    NCH = B // CH   # 4

    n_copy = [0]

    def strip_copy(dst, src):
        if n_copy[0] % 2 == 0:
            nc.scalar.copy(out=dst, in_=src)
        else:
            nc.vector.tensor_copy(out=dst, in_=src)
        n_copy[0] += 1

    with tile.TileContext(nc) as tc:
        with (
            tc.tile_pool(name="const", bufs=1) as const_pool,
            tc.tile_pool(name="idx", bufs=1) as idx_pool,
            tc.tile_pool(name="g", bufs=4) as g_pool,
            tc.tile_pool(name="gh", bufs=4) as gh_pool,
            tc.tile_pool(name="fops", bufs=1) as fops_pool,
            tc.tile_pool(name="bops", bufs=2) as bops_pool,
            tc.tile_pool(name="strip", bufs=6) as strip_pool,
            tc.tile_pool(name="tpsum", bufs=2, space="PSUM") as tpsum_pool,
            tc.tile_pool(name="mpsum", bufs=3, space="PSUM") as mpsum_pool,
        ):
            identity = const_pool.tile([P, P], bf16)
            make_identity(nc, identity[:])

            idxf = idx_pool.tile([P, GF], i32, tag="idxf")
            idxb = idx_pool.tile([P, GB], i32, tag="idxb")
            nc.sync.dma_start(idxf[:], idxf_d[:])
            nc.sync.dma_start(idxb[:], idxb_d[:])

            def gather_slice(table, idx_tile, g0):
                # 8 indirect DMAs into one [128, 512] tile (multi-index
                # offset APs are not supported by the SWDGE firmware: each
                # partition reads offsets from a sliding window across
                # partitions, so only [128, 1] offsets gather correctly).
                t = g_pool.tile([P, SG * D], f32, tag="g")
                for r in range(SG):
                    nc.gpsimd.indirect_dma_start(
                        out=t[:, r * D:(r + 1) * D],
                        out_offset=None,
                        in_=table[:],
                        in_offset=bass.IndirectOffsetOnAxis(
                            ap=idx_tile[:, g0 + r:g0 + r + 1], axis=0
                        ),
                    )
                return t

            def prep_operand(t, dest):
                # cast f32 -> bf16 off the critical copy engines
                th = gh_pool.tile([P, SG * D], bf16, tag="gh")
                nc.gpsimd.tensor_copy(out=th[:], in_=t[:])
                pt = tpsum_pool.tile([D, SG * P], bf16, tag="pt")
                for r in range(SG):
                    nc.tensor.transpose(
                        out=pt[:, r * P:(r + 1) * P],
                        in_=th[:, r * D:(r + 1) * D],
                        identity=identity[:],
                    )
                nc.scalar.copy(out=dest, in_=pt[:])

            # Forward operand: [64, 1024] bf16.
            fwdT = fops_pool.tile([D, M], bf16, tag="fwdT")
            fg = gather_slice(wf, idxf, 0)
            prep_operand(fg, fwdT[:, :])

            # Column-chunk-outer pipeline over the backward reprs.
            for ch in range(NCH):
                bt = bops_pool.tile([D, CH], bf16, tag="bt")
                for h in range(2):
                    bg = gather_slice(wb, idxb, ch * (CH // P) + h * SG)
                    prep_operand(bg, bt[:, h * SG * P:(h + 1) * SG * P])

                for i in range(GF):  # 8 output row tiles
                    strip = strip_pool.tile([P, CH], bf16, tag="strip")
                    for hh in range(2):
                        ps = mpsum_pool.tile([P, 1024], f32, tag="ps")
                        for q in range(2):
                            nc.tensor.matmul(
                                out=ps[:, q * 512:(q + 1) * 512],
                                lhsT=fwdT[:, i * P:(i + 1) * P],
                                rhs=bt[:, hh * 1024 + q * 512:
                                       hh * 1024 + (q + 1) * 512],
                                start=True,
                                stop=True,
                            )
                        strip_copy(strip[:, hh * 1024:(hh + 1) * 1024], ps[:])
                    nc.sync.dma_start(
                        out_d[i * P:(i + 1) * P, ch * CH:(ch + 1) * CH],
                        strip[:],
                    )

    nc.compile()
    return nc


def _get_nc():
    if "nc" not in _CACHE:
        _CACHE["nc"] = _build_nc()
    return _CACHE["nc"]


def _ravel_clip(obs, act):
    o = np.clip(obs.astype(np.int64), 0, NUM_OBS - 1)
    a = np.clip(act.astype(np.int64), 0, NUM_ACT - 1)
    return (o * NUM_ACT + a).astype(np.int32)


def make_in_maps(observations, actions, future_observations, future_actions,
                 W_f, W_b):
    fwd_idx = _ravel_clip(np.asarray(observations), np.asarray(actions))
    bwd_idx = _ravel_clip(np.asarray(future_observations),
                          np.asarray(future_actions))
    wf = np.ascontiguousarray(np.asarray(W_f, dtype=np.float32))
    wb = np.ascontiguousarray(np.asarray(W_b, dtype=np.float32))
    # [p, g] = idx[g*128 + p]
    idxb = np.ascontiguousarray(bwd_idx.reshape(B // P, P).T)
    in_maps = []
    for c in range(N_CORES):
        idxf = np.ascontiguousarray(
            fwd_idx[c * M:(c + 1) * M].reshape(M // P, P).T
        )
        in_maps.append({"wf": wf, "wb": wb, "idxf": idxf, "idxb": idxb})
    return in_maps


def kernel(**inputs):
    from concourse.bass_utils import run_bass_kernel_spmd

    in_maps = make_in_maps(
        inputs["observations"], inputs["actions"],
        inputs["future_observations"], inputs["future_actions"],
        inputs["W_f"], inputs["W_b"],
    )
    res = run_bass_kernel_spmd(_get_nc(), in_maps, core_ids=list(range(N_CORES)),
                               trace=TRACE)
    globals()["LAST_RESULT"] = res
    return np.concatenate(
        [res.results[c]["out"] for c in range(N_CORES)], axis=0
    ).astype(np.float32)


# revision 6
# speedup vs baseline: 1.2866x; 1.2866x over previous
"""FBCritic embedding-lookup kernel for 8 Trainium2 NeuronCores.

Math (reference):
    fwd_idx = clip(obs)*10 + clip(act)            # [8192]
    bwd_idx = clip(fobs)*10 + clip(fact)          # [8192]
    F = W_f[fwd_idx]                              # [8192, 64]
    B = W_b[bwd_idx]                              # [8192, 64]
    out = F @ B.T                                 # [8192, 8192]

Sharding: 4x2 grid over the output. Core c = (r, s) computes rows
[r*2048, (r+1)*2048) x cols [s*4096, (s+1)*4096). The 2D grid cuts the
number of gathered table rows per core from 9216 to 6144: indirect-DMA
dispatch on gpsimd costs ~1.1us per 128-row instruction regardless of
transfer size (SWDGE descriptor generation), and is the serial bottleneck,
so fewer gathers directly shortens the critical path.

Per core: 48 single-offset indirect DMAs gather 6144 f32 table rows
(multi-offset APs are unsupported: SWDGE reads per-partition offsets from
a 4B-strided sliding window, so only [128, 1] offset APs gather
correctly). Vector/scalar casts to bf16, PE transposes emit bf16 PSUM
tiles (transpose output dtype follows its input), and [64, 512] bf16
operand quarters copy to SBUF in the DVE 16-bit fast path. bf16 matmuls
accumulate f32 into PSUM; strip evacuation (PSUM f32 -> SBUF bf16)
alternates between vector and scalar, and [128, 2048] bf16 strips (4KB
per partition row) stream out on the sync HWDGE queue. Output lands in
HBM as bf16 (halving the dominant write traffic) and is upcast to f32
host-side during unsharding.
"""

import numpy as np

NUM_OBS = 100000
NUM_ACT = 10
V = NUM_OBS * NUM_ACT  # 1_000_000 table rows
D = 64                 # repr dim
B = 8192               # batch
N_CORES = 8
RS = 4                 # row shards
CS = 2                 # col shards
MR = B // RS           # 2048 output rows per core
NC = B // CS           # 4096 output cols per core
P = 128                # partitions

_CACHE = {}
TRACE = False
LAST_RESULT = None


def _build_nc():
    import concourse.bass as bass
    import concourse.tile as tile
    from concourse import bacc, mybir
    from concourse.masks import make_identity

    f32 = mybir.dt.float32
    bf16 = mybir.dt.bfloat16
    i32 = mybir.dt.int32

    nc = bacc.Bacc("TRN2", target_bir_lowering=False, debug=False)

    wf = nc.dram_tensor("wf", [V, D], f32, kind="ExternalInput").ap()
    wb = nc.dram_tensor("wb", [V, D], f32, kind="ExternalInput").ap()
    idxf_d = nc.dram_tensor("idxf", [P, MR // P], i32, kind="ExternalInput").ap()
    idxb_d = nc.dram_tensor("idxb", [P, NC // P], i32, kind="ExternalInput").ap()
    out_d = nc.dram_tensor("out", [MR, NC], bf16, kind="ExternalOutput").ap()

    GF = MR // P    # 16 forward 128-row groups
    GB = NC // P    # 32 backward 128-row groups
    QG = 4          # groups per operand quarter ([64, 512] bf16)
    FQ = GF // QG   # 4 forward quarters
    BQ = GB // QG   # 8 backward quarters
    CH = 2048       # output strip width (4KB bf16 rows)
    NCH = NC // CH  # 2 column chunks

    n_copy = [0]
    n_prep = [0]

    def strip_copy(dst, src):
        if n_copy[0] % 2 == 0:
            nc.scalar.copy(out=dst, in_=src)
        else:
            nc.vector.tensor_copy(out=dst, in_=src)
        n_copy[0] += 1

    with tile.TileContext(nc) as tc:
        with (
            tc.tile_pool(name="const", bufs=1) as const_pool,
            tc.tile_pool(name="idx", bufs=1) as idx_pool,
            tc.tile_pool(name="g", bufs=6) as g_pool,
            tc.tile_pool(name="gh", bufs=4) as gh_pool,
            tc.tile_pool(name="ops", bufs=1) as ops_pool,
            tc.tile_pool(name="strip", bufs=6) as strip_pool,
            tc.tile_pool(name="tpsum", bufs=2, space="PSUM") as tpsum_pool,
            tc.tile_pool(name="mpsum", bufs=3, space="PSUM") as mpsum_pool,
        ):
            identity = const_pool.tile([P, P], bf16)
            make_identity(nc, identity[:])

            idxf = idx_pool.tile([P, GF], i32, tag="idxf")
            idxb = idx_pool.tile([P, GB], i32, tag="idxb")
            nc.sync.dma_start(idxf[:], idxf_d[:])
            nc.sync.dma_start(idxb[:], idxb_d[:])

            fwdT = ops_pool.tile([D, MR], bf16, tag="fwdT")
            btall = ops_pool.tile([D, NC], bf16, tag="btall")

            def prep_quarter(table, idx_tile, g0, dest):
                # 4 single-offset indirect gathers -> cast -> 4 PE
                # transposes -> one bf16 PSUM->SBUF operand copy.
                t = g_pool.tile([P, QG * D], f32, tag="g")
                for r in range(QG):
                    nc.gpsimd.indirect_dma_start(
                        out=t[:, r * D:(r + 1) * D],
                        out_offset=None,
                        in_=table[:],
                        in_offset=bass.IndirectOffsetOnAxis(
                            ap=idx_tile[:, g0 + r:g0 + r + 1], axis=0
                        ),
                    )
                th = gh_pool.tile([P, QG * D], bf16, tag="gh")
                cast_eng = nc.vector if n_prep[0] % 2 == 0 else nc.scalar
                if cast_eng is nc.vector:
                    cast_eng.tensor_copy(out=th[:], in_=t[:])
                else:
                    cast_eng.copy(out=th[:], in_=t[:])
                pt = tpsum_pool.tile([D, QG * P], bf16, tag="pt")
                for r in range(QG):
                    nc.tensor.transpose(
                        out=pt[:, r * P:(r + 1) * P],
                        in_=th[:, r * D:(r + 1) * D],
                        identity=identity[:],
                    )
                if n_prep[0] % 2 == 0:
                    nc.scalar.copy(out=dest, in_=pt[:])
                else:
                    nc.vector.tensor_copy(out=dest, in_=pt[:])
                n_prep[0] += 1

            # Gather order: bwd chunk-0 quarters, then fwd quarters (rows
            # unlock progressively), then bwd chunk-1 quarters.
            for q in range(QG):
                prep_quarter(wb, idxb, q * QG, btall[:, q * 512:(q + 1) * 512])
            prep_quarter(wf, idxf, 0, fwdT[:, 0:512])
            for ch in range(NCH):
                if ch == 1:
                    for q in range(QG, BQ):
                        prep_quarter(wb, idxb, q * QG,
                                     btall[:, q * 512:(q + 1) * 512])
                for i in range(GF):  # 16 output row tiles
                    if ch == 0 and i in (4, 8, 12):
                        prep_quarter(wf, idxf, i,
                                     fwdT[:, i * P:(i + 4) * P])
                    strip = strip_pool.tile([P, CH], bf16, tag="strip")
                    for hh in range(2):
                        ps = mpsum_pool.tile([P, 1024], f32, tag="ps")
                        for q in range(2):
                            j = ch * CH + hh * 1024 + q * 512
                            nc.tensor.matmul(
                                out=ps[:, q * 512:(q + 1) * 512],
                                lhsT=fwdT[:, i * P:(i + 1) * P],
                                rhs=btall[:, j:j + 512],
                                start=True,
                                stop=True,
                            )
                        strip_copy(strip[:, hh * 1024:(hh + 1) * 1024], ps[:])
                    nc.sync.dma_start(
                        out_d[i * P:(i + 1) * P, ch * CH:(ch + 1) * CH],
                        strip[:],
                    )

    nc.compile()
    return nc


def _get_nc():
    if "nc" not in _CACHE:
        _CACHE["nc"] = _build_nc()
    return _CACHE["nc"]


def _ravel_clip(obs, act):
    o = np.clip(obs.astype(np.int64), 0, NUM_OBS - 1)
    a = np.clip(act.astype(np.int64), 0, NUM_ACT - 1)
    return (o * NUM_ACT + a).astype(np.int32)


def make_in_maps(observations, actions, future_observations, future_actions,
                 W_f, W_b):
    fwd_idx = _ravel_clip(np.asarray(observations), np.asarray(actions))
    bwd_idx = _ravel_clip(np.asarray(future_observations),
                          np.asarray(future_actions))
    wf = np.ascontiguousarray(np.asarray(W_f, dtype=np.float32))
    wb = np.ascontiguousarray(np.asarray(W_b, dtype=np.float32))
    in_maps = []
    for c in range(N_CORES):
        r, s = divmod(c, CS)
        # [p, g] = idx[g*128 + p]
        idxf = np.ascontiguousarray(
            fwd_idx[r * MR:(r + 1) * MR].reshape(MR // P, P).T
        )
        idxb = np.ascontiguousarray(
            bwd_idx[s * NC:(s + 1) * NC].reshape(NC // P, P).T
        )
        in_maps.append({"wf": wf, "wb": wb, "idxf": idxf, "idxb": idxb})
    return in_maps


def kernel(**inputs):
    from concourse.bass_utils import run_bass_kernel_spmd

    in_maps = make_in_maps(
        inputs["observations"], inputs["actions"],
        inputs["future_observations"], inputs["future_actions"],
        inputs["W_f"], inputs["W_b"],
    )
    res = run_bass_kernel_spmd(_get_nc(), in_maps, core_ids=list(range(N_CORES)),
                               trace=TRACE)
    globals()["LAST_RESULT"] = res
    out = np.empty((B, B), dtype=np.float32)
    for c in range(N_CORES):
        r, s = divmod(c, CS)
        out[r * MR:(r + 1) * MR, s * NC:(s + 1) * NC] = res.results[c]["out"]
    return out


# revision 8
# speedup vs baseline: 1.3207x; 1.0265x over previous
"""FBCritic embedding-lookup kernel for 8 Trainium2 NeuronCores.

Math (reference):
    fwd_idx = clip(obs)*10 + clip(act)            # [8192]
    bwd_idx = clip(fobs)*10 + clip(fact)          # [8192]
    F = W_f[fwd_idx]                              # [8192, 64]
    B = W_b[bwd_idx]                              # [8192, 64]
    out = F @ B.T                                 # [8192, 8192]

Sharding: 4x2 grid over the output. Core c = (r, s) computes rows
[r*2048, (r+1)*2048) x cols [s*4096, (s+1)*4096). The 2D grid cuts the
number of gathered table rows per core from 9216 to 6144: indirect-DMA
dispatch on gpsimd costs ~1.1us per 128-row instruction regardless of
transfer size (SWDGE descriptor generation), and is the serial bottleneck,
so fewer gathers directly shortens the critical path.

Per core: 48 single-offset indirect DMAs gather 6144 f32 table rows
(multi-offset APs are unsupported: SWDGE reads per-partition offsets from
a 4B-strided sliding window, so only [128, 1] offset APs gather
correctly). Vector/scalar casts to bf16, PE transposes emit bf16 PSUM
tiles (transpose output dtype follows its input), and [64, 512] bf16
operand quarters copy to SBUF in the DVE 16-bit fast path. bf16 matmuls
accumulate f32 into PSUM; strip evacuation (PSUM f32 -> SBUF bf16)
alternates between vector and scalar, and [128, 2048] bf16 strips (4KB
per partition row) stream out on the sync HWDGE queue. Output lands in
HBM as bf16 (halving the dominant write traffic) and is upcast to f32
host-side during unsharding.
"""

import numpy as np

NUM_OBS = 100000
NUM_ACT = 10
V = NUM_OBS * NUM_ACT  # 1_000_000 table rows
D = 64                 # repr dim
B = 8192               # batch
N_CORES = 8
RS = 4                 # row shards
CS = 2                 # col shards
MR = B // RS           # 2048 output rows per core
NC = B // CS           # 4096 output cols per core
P = 128                # partitions

_CACHE = {}
TRACE = False
LAST_RESULT = None


def _build_nc():
    import concourse.bass as bass
    import concourse.tile as tile
    from concourse import bacc, mybir
    from concourse.masks import make_identity

    f32 = mybir.dt.float32
    bf16 = mybir.dt.bfloat16
    i32 = mybir.dt.int32

    nc = bacc.Bacc("TRN2", target_bir_lowering=False, debug=False)

    wf = nc.dram_tensor("wf", [V, D], f32, kind="ExternalInput").ap()
    wb = nc.dram_tensor("wb", [V, D], f32, kind="ExternalInput").ap()
    idxf_d = nc.dram_tensor("idxf", [P, MR // P], i32, kind="ExternalInput").ap()
    idxb_d = nc.dram_tensor("idxb", [P, NC // P], i32, kind="ExternalInput").ap()
    out_d = nc.dram_tensor("out", [MR, NC], bf16, kind="ExternalOutput").ap()

    GF = MR // P    # 16 forward 128-row groups
    GB = NC // P    # 32 backward 128-row groups
    QG = 4          # groups per operand quarter ([64, 512] bf16)
    FQ = GF // QG   # 4 forward quarters
    BQ = GB // QG   # 8 backward quarters
    CH = 1024       # output strip width (2KB bf16 rows)
    NCP = NC // CH  # 4 column pairs (2 bwd quarters each)

    n_copy = [0]
    n_prep = [0]

    def strip_copy(dst, src):
        if n_copy[0] % 2 == 0:
            nc.scalar.copy(out=dst, in_=src)
        else:
            nc.vector.tensor_copy(out=dst, in_=src)
        n_copy[0] += 1

    with tile.TileContext(nc) as tc:
        with (
            tc.tile_pool(name="const", bufs=1) as const_pool,
            tc.tile_pool(name="idx", bufs=1) as idx_pool,
            tc.tile_pool(name="g", bufs=6) as g_pool,
            tc.tile_pool(name="gh", bufs=4) as gh_pool,
            tc.tile_pool(name="ops", bufs=1) as ops_pool,
            tc.tile_pool(name="strip", bufs=6) as strip_pool,
            tc.tile_pool(name="tpsum", bufs=2, space="PSUM") as tpsum_pool,
            tc.tile_pool(name="mpsum", bufs=3, space="PSUM") as mpsum_pool,
        ):
            identity = const_pool.tile([P, P], bf16)
            make_identity(nc, identity[:])

            idxf = idx_pool.tile([P, GF], i32, tag="idxf")
            idxb = idx_pool.tile([P, GB], i32, tag="idxb")
            nc.sync.dma_start(idxf[:], idxf_d[:])
            nc.sync.dma_start(idxb[:], idxb_d[:])

            fwdT = ops_pool.tile([D, MR], bf16, tag="fwdT")
            btall = ops_pool.tile([D, NC], bf16, tag="btall")

            def prep_quarter(table, idx_tile, g0, dest):
                # 4 single-offset indirect gathers -> cast -> 4 PE
                # transposes -> one bf16 PSUM->SBUF operand copy.
                t = g_pool.tile([P, QG * D], f32, tag="g")
                for r in range(QG):
                    nc.gpsimd.indirect_dma_start(
                        out=t[:, r * D:(r + 1) * D],
                        out_offset=None,
                        in_=table[:],
                        in_offset=bass.IndirectOffsetOnAxis(
                            ap=idx_tile[:, g0 + r:g0 + r + 1], axis=0
                        ),
                    )
                th = gh_pool.tile([P, QG * D], bf16, tag="gh")
                cast_eng = nc.vector if n_prep[0] % 2 == 0 else nc.scalar
                if cast_eng is nc.vector:
                    cast_eng.tensor_copy(out=th[:], in_=t[:])
                else:
                    cast_eng.copy(out=th[:], in_=t[:])
                pt = tpsum_pool.tile([D, QG * P], bf16, tag="pt")
                for r in range(QG):
                    nc.tensor.transpose(
                        out=pt[:, r * P:(r + 1) * P],
                        in_=th[:, r * D:(r + 1) * D],
                        identity=identity[:],
                    )
                if n_prep[0] % 2 == 0:
                    nc.scalar.copy(out=dest, in_=pt[:])
                else:
                    nc.vector.tensor_copy(out=dest, in_=pt[:])
                n_prep[0] += 1

            def emit_strip(i, cp):
                strip = strip_pool.tile([P, CH], bf16, tag="strip")
                ps = mpsum_pool.tile([P, CH], f32, tag="ps")
                for q in range(2):
                    j = cp * CH + q * 512
                    nc.tensor.matmul(
                        out=ps[:, q * 512:(q + 1) * 512],
                        lhsT=fwdT[:, i * P:(i + 1) * P],
                        rhs=btall[:, j:j + 512],
                        start=True,
                        stop=True,
                    )
                strip_copy(strip[:], ps[:])
                nc.sync.dma_start(
                    out_d[i * P:(i + 1) * P, cp * CH:(cp + 1) * CH],
                    strip[:],
                )

            # Arrival-ordered schedule: interleave bwd column-pair and fwd
            # quarter gathers, and emit each [128, 1024] strip as soon as
            # both of its operand quarters have been prepped, so compute
            # and output DMA track the serial gather dispatch on gpsimd
            # instead of piling up behind it.
            order = [("b", 0), ("b", 1), ("f", 0), ("b", 2), ("b", 3),
                     ("f", 1), ("b", 4), ("b", 5), ("f", 2), ("b", 6),
                     ("b", 7), ("f", 3)]
            rows_ready = 0
            cps_ready = 0
            emitted = set()
            for kind, k in order:
                if kind == "b":
                    prep_quarter(wb, idxb, k * QG,
                                 btall[:, k * 512:(k + 1) * 512])
                    cps_ready = (k + 1) // 2
                else:
                    prep_quarter(wf, idxf, k * QG,
                                 fwdT[:, k * 512:(k + 1) * 512])
                    rows_ready = (k + 1) * QG
                for cp in range(cps_ready):
                    for i in range(rows_ready):
                        if (i, cp) not in emitted:
                            emitted.add((i, cp))
                            emit_strip(i, cp)

    nc.compile()
    return nc


def _get_nc():
    if "nc" not in _CACHE:
        _CACHE["nc"] = _build_nc()
    return _CACHE["nc"]


def _ravel_clip(obs, act):
    o = np.clip(obs.astype(np.int64), 0, NUM_OBS - 1)
    a = np.clip(act.astype(np.int64), 0, NUM_ACT - 1)
    return (o * NUM_ACT + a).astype(np.int32)


def make_in_maps(observations, actions, future_observations, future_actions,
                 W_f, W_b):
    fwd_idx = _ravel_clip(np.asarray(observations), np.asarray(actions))
    bwd_idx = _ravel_clip(np.asarray(future_observations),
                          np.asarray(future_actions))
    wf = np.ascontiguousarray(np.asarray(W_f, dtype=np.float32))
    wb = np.ascontiguousarray(np.asarray(W_b, dtype=np.float32))
    in_maps = []
    for c in range(N_CORES):
        r, s = divmod(c, CS)
        # [p, g] = idx[g*128 + p]
        idxf = np.ascontiguousarray(
            fwd_idx[r * MR:(r + 1) * MR].reshape(MR // P, P).T
        )
        idxb = np.ascontiguousarray(
            bwd_idx[s * NC:(s + 1) * NC].reshape(NC // P, P).T
        )
        in_maps.append({"wf": wf, "wb": wb, "idxf": idxf, "idxb": idxb})
    return in_maps


def kernel(**inputs):
    from concourse.bass_utils import run_bass_kernel_spmd

    in_maps = make_in_maps(
        inputs["observations"], inputs["actions"],
        inputs["future_observations"], inputs["future_actions"],
        inputs["W_f"], inputs["W_b"],
    )
    res = run_bass_kernel_spmd(_get_nc(), in_maps, core_ids=list(range(N_CORES)),
                               trace=TRACE)
    globals()["LAST_RESULT"] = res
    out = np.empty((B, B), dtype=np.float32)
    for c in range(N_CORES):
        r, s = divmod(c, CS)
        out[r * MR:(r + 1) * MR, s * NC:(s + 1) * NC] = res.results[c]["out"]
    return out


# revision 11
# speedup vs baseline: 1.3328x; 1.0092x over previous
"""FBCritic embedding-lookup kernel for 8 Trainium2 NeuronCores.

Math (reference):
    fwd_idx = clip(obs)*10 + clip(act)            # [8192]
    bwd_idx = clip(fobs)*10 + clip(fact)          # [8192]
    F = W_f[fwd_idx]                              # [8192, 64]
    B = W_b[bwd_idx]                              # [8192, 64]
    out = F @ B.T                                 # [8192, 8192]

Sharding: 4x2 grid over the output. Core c = (r, s) computes rows
[r*2048, (r+1)*2048) x cols [s*4096, (s+1)*4096). The 2D grid cuts the
number of gathered table rows per core from 9216 to 6144: indirect-DMA
dispatch on gpsimd costs ~1.1us per 128-row instruction regardless of
transfer size (SWDGE descriptor generation), and is the serial bottleneck,
so fewer gathers directly shortens the critical path.

Per core: 48 single-offset indirect DMAs gather 6144 f32 table rows
(multi-offset APs are unsupported: SWDGE reads per-partition offsets from
a 4B-strided sliding window, so only [128, 1] offset APs gather
correctly). Vector/scalar casts to bf16, PE transposes emit bf16 PSUM
tiles (transpose output dtype follows its input), and [64, 512] bf16
operand quarters copy to SBUF in the DVE 16-bit fast path. bf16 matmuls
accumulate f32 into PSUM; strip evacuation (PSUM f32 -> SBUF bf16)
alternates between vector and scalar, and [128, 2048] bf16 strips (4KB
per partition row) stream out on the sync HWDGE queue. Output lands in
HBM as bf16 (halving the dominant write traffic) and is upcast to f32
host-side during unsharding.
"""

import numpy as np

NUM_OBS = 100000
NUM_ACT = 10
V = NUM_OBS * NUM_ACT  # 1_000_000 table rows
D = 64                 # repr dim
B = 8192               # batch
N_CORES = 8
RS = 4                 # row shards
CS = 2                 # col shards
MR = B // RS           # 2048 output rows per core
NC = B // CS           # 4096 output cols per core
P = 128                # partitions

_CACHE = {}
TRACE = False
LAST_RESULT = None


def _build_nc():
    import concourse.bass as bass
    import concourse.tile as tile
    from concourse import bacc, mybir
    from concourse.masks import make_identity

    f32 = mybir.dt.float32
    bf16 = mybir.dt.bfloat16
    i32 = mybir.dt.int32

    nc = bacc.Bacc("TRN2", target_bir_lowering=False, debug=False)

    wf = nc.dram_tensor("wf", [V, D], f32, kind="ExternalInput").ap()
    wb = nc.dram_tensor("wb", [V, D], f32, kind="ExternalInput").ap()
    idxf_d = nc.dram_tensor("idxf", [P, MR // P], i32, kind="ExternalInput").ap()
    idxb_d = nc.dram_tensor("idxb", [P, NC // P], i32, kind="ExternalInput").ap()
    out_d = nc.dram_tensor("out", [MR, NC], bf16, kind="ExternalOutput").ap()

    GF = MR // P    # 16 forward 128-row groups
    GB = NC // P    # 32 backward 128-row groups
    QG = 4          # groups per operand quarter ([64, 512] bf16)
    FQ = GF // QG   # 4 forward quarters
    BQ = GB // QG   # 8 backward quarters
    CH = 1024       # output strip width (2KB bf16 rows)
    NCP = NC // CH  # 4 column pairs (2 bwd quarters each)

    n_copy = [0]
    n_prep = [0]

    def strip_copy(dst, src):
        if n_copy[0] % 2 == 0:
            nc.scalar.copy(out=dst, in_=src)
        else:
            nc.vector.tensor_copy(out=dst, in_=src)
        n_copy[0] += 1

    with tile.TileContext(nc) as tc:
        with (
            tc.tile_pool(name="const", bufs=1) as const_pool,
            tc.tile_pool(name="idx", bufs=1) as idx_pool,
            tc.tile_pool(name="g", bufs=6) as g_pool,
            tc.tile_pool(name="gh", bufs=4) as gh_pool,
            tc.tile_pool(name="ops", bufs=1) as ops_pool,
            tc.tile_pool(name="strip", bufs=6) as strip_pool,
            tc.tile_pool(name="tpsum", bufs=2, space="PSUM") as tpsum_pool,
            tc.tile_pool(name="mpsum", bufs=3, space="PSUM") as mpsum_pool,
        ):
            identity = const_pool.tile([P, P], bf16)
            make_identity(nc, identity[:])

            idxf = idx_pool.tile([P, GF], i32, tag="idxf")
            idxb = idx_pool.tile([P, GB], i32, tag="idxb")
            nc.sync.dma_start(idxf[:], idxf_d[:])
            nc.sync.dma_start(idxb[:], idxb_d[:])

            # Operands duplicated across partitions 0-63 / 64-127 so each
            # strip's two matmuls run concurrently in PE row-groups (0,0)
            # and (64,0) - a single K=64 matmul leaves half the array idle
            # and the HAM clock-gate then holds the PE at reduced clock.
            fwdT = ops_pool.tile([P, MR], bf16, tag="fwdT")
            btall = ops_pool.tile([P, NC], bf16, tag="btall")

            def prep_quarter(table, idx_tile, g0, dest):
                # 4 single-offset indirect gathers -> cast -> 4 PE
                # transposes -> one bf16 PSUM->SBUF operand copy.
                t = g_pool.tile([P, QG * D], f32, tag="g")
                for r in range(QG):
                    nc.gpsimd.indirect_dma_start(
                        out=t[:, r * D:(r + 1) * D],
                        out_offset=None,
                        in_=table[:],
                        in_offset=bass.IndirectOffsetOnAxis(
                            ap=idx_tile[:, g0 + r:g0 + r + 1], axis=0
                        ),
                    )
                th = gh_pool.tile([P, QG * D], bf16, tag="gh")
                cast_eng = nc.vector if n_prep[0] % 2 == 0 else nc.scalar
                if cast_eng is nc.vector:
                    cast_eng.tensor_copy(out=th[:], in_=t[:])
                else:
                    cast_eng.copy(out=th[:], in_=t[:])
                pt = tpsum_pool.tile([P, QG * P], bf16, tag="pt")
                for r in range(QG):
                    # Concurrent col-group pair: same source, PSUM
                    # partitions 0-63 and 64-127.
                    nc.tensor.transpose(
                        out=pt[0:D, r * P:(r + 1) * P],
                        in_=th[:, r * D:(r + 1) * D],
                        identity=identity[:],
                        tile_position=(0, 0),
                    )
                    nc.tensor.transpose(
                        out=pt[D:2 * D, r * P:(r + 1) * P],
                        in_=th[:, r * D:(r + 1) * D],
                        identity=identity[:],
                        tile_position=(0, D),
                    )
                if n_prep[0] % 2 == 0:
                    nc.scalar.copy(out=dest, in_=pt[:])
                else:
                    nc.vector.tensor_copy(out=dest, in_=pt[:])
                n_prep[0] += 1

            def emit_strip(i, cp):
                strip = strip_pool.tile([P, CH], bf16, tag="strip")
                ps = mpsum_pool.tile([P, CH], f32, tag="ps")
                for q in range(2):
                    j = cp * CH + q * 512
                    nc.tensor.matmul(
                        out=ps[:, q * 512:(q + 1) * 512],
                        lhsT=fwdT[q * D:(q + 1) * D, i * P:(i + 1) * P],
                        rhs=btall[q * D:(q + 1) * D, j:j + 512],
                        start=True,
                        stop=True,
                        tile_position=(q * D, 0),
                    )
                strip_copy(strip[:], ps[:])
                nc.sync.dma_start(
                    out_d[i * P:(i + 1) * P, cp * CH:(cp + 1) * CH],
                    strip[:],
                )

            # Arrival-ordered schedule: interleave bwd column-pair and fwd
            # quarter gathers, and emit each [128, 1024] strip as soon as
            # both of its operand quarters have been prepped, so compute
            # and output DMA track the serial gather dispatch on gpsimd
            # instead of piling up behind it.
            order = [("b", 0), ("b", 1), ("f", 0), ("b", 2), ("b", 3),
                     ("f", 1), ("b", 4), ("b", 5), ("f", 2), ("b", 6),
                     ("b", 7), ("f", 3)]
            rows_ready = 0
            cps_ready = 0
            emitted = set()
            for kind, k in order:
                if kind == "b":
                    prep_quarter(wb, idxb, k * QG,
                                 btall[:, k * 512:(k + 1) * 512])
                    cps_ready = (k + 1) // 2
                else:
                    prep_quarter(wf, idxf, k * QG,
                                 fwdT[:, k * 512:(k + 1) * 512])
                    rows_ready = (k + 1) * QG
                for cp in range(cps_ready):
                    for i in range(rows_ready):
                        if (i, cp) not in emitted:
                            emitted.add((i, cp))
                            emit_strip(i, cp)

    nc.compile()
    return nc


def _get_nc():
    if "nc" not in _CACHE:
        _CACHE["nc"] = _build_nc()
    return _CACHE["nc"]


def _ravel_clip(obs, act):
    o = np.clip(obs.astype(np.int64), 0, NUM_OBS - 1)
    a = np.clip(act.astype(np.int64), 0, NUM_ACT - 1)
    return (o * NUM_ACT + a).astype(np.int32)


def make_in_maps(observations, actions, future_observations, future_actions,
                 W_f, W_b):
    fwd_idx = _ravel_clip(np.asarray(observations), np.asarray(actions))
    bwd_idx = _ravel_clip(np.asarray(future_observations),
                          np.asarray(future_actions))
    wf = np.ascontiguousarray(np.asarray(W_f, dtype=np.float32))
    wb = np.ascontiguousarray(np.asarray(W_b, dtype=np.float32))
    in_maps = []
    for c in range(N_CORES):
        r, s = divmod(c, CS)
        # [p, g] = idx[g*128 + p]
        idxf = np.ascontiguousarray(
            fwd_idx[r * MR:(r + 1) * MR].reshape(MR // P, P).T
        )
        idxb = np.ascontiguousarray(
            bwd_idx[s * NC:(s + 1) * NC].reshape(NC // P, P).T
        )
        in_maps.append({"wf": wf, "wb": wb, "idxf": idxf, "idxb": idxb})
    return in_maps


def kernel(**inputs):
    from concourse.bass_utils import run_bass_kernel_spmd

    in_maps = make_in_maps(
        inputs["observations"], inputs["actions"],
        inputs["future_observations"], inputs["future_actions"],
        inputs["W_f"], inputs["W_b"],
    )
    res = run_bass_kernel_spmd(_get_nc(), in_maps, core_ids=list(range(N_CORES)),
                               trace=TRACE)
    globals()["LAST_RESULT"] = res
    out = np.empty((B, B), dtype=np.float32)
    for c in range(N_CORES):
        r, s = divmod(c, CS)
        out[r * MR:(r + 1) * MR, s * NC:(s + 1) * NC] = res.results[c]["out"]
    return out
